# revision 25
# baseline (speedup 1.0000x reference)
"""Trainium2 Bass kernel for nn_DecoderBlock (B=1, S=2048, D=1024, H=16, DQ=64, DM=4096).

Strategy (8 NeuronCores, one chip):
  - Attention tensor-parallel over heads (core c owns heads {2c, 2c+1});
    per-head projections fused with QKV: weff_x = H_x @ W_x computed on device.
    Scores kept TRANSPOSED [keys, queries] so softmax's reduction rides the
    o-matmul contraction (row-of-ones denominator trick).
  - Causal skip: scores/exp/o computed only for key tiles at-or-below the
    query block; the diagonal-tile mask is generated on device (affine_select),
    no mask DMA at all.
  - bf16 everywhere off the residual spine (weights, E, q/k/v, exp weights,
    MLP hidden) to halve HBM traffic and unlock DVE 4x elementwise; residuals
    and LN statistics stay fp32.
  - One AllToAll flips heads-sharded -> token-sharded; post-attention
    (W_O, LN1, MLP, LN2) data-parallel over tokens in [d, s] layout,
    LayerNorm stats via ones-matmuls.
  - All weight DMAs issued up front so the DMA engine streams continuously
    behind compute (l1t prefetched during attention).
"""

import contextlib

import numpy as np
import ml_dtypes

BF16 = ml_dtypes.bfloat16

B, S_FULL, D, H, DQ, DM = 1, 2048, 1024, 16, 64, 4096
NC = 8          # cores
P = 128         # partitions
HPC = H // NC   # heads per core (2)
EPC = HPC * DQ  # per-core attention width (128)
DK = D // P     # d-model chunks (8)
MK = DM // P    # mlp chunks (32)
QB = 512        # query-block width in attention
KPQ = QB // P   # key tiles per query block (4)
EPS = 1e-5


def _body(tc, io, S, use_collective=True, stop_after=None):
    import concourse.bass as bass
    import concourse.mybir as mybir
    from concourse.masks import make_identity

    nc = tc.nc
    fp32 = mybir.dt.float32
    bf16 = mybir.dt.bfloat16
    f32r = mybir.dt.float32r
    Exp = mybir.ActivationFunctionType.Exp
    Relu = mybir.ActivationFunctionType.Relu
    Sqrt = mybir.ActivationFunctionType.Sqrt
    Copy = mybir.ActivationFunctionType.Copy
    Ident = mybir.ActivationFunctionType.Identity
    sub_op = mybir.AluOpType.subtract
    mult_op = mybir.AluOpType.mult
    add_op = mybir.AluOpType.add

    SL = S // NC                 # tokens per core (256)
    TT = S // P                  # key tiles (16)
    NQB = S // QB                # query blocks (4)
    MW = 896                     # mask width: QB + (KPQ-1)*P
    ts = bass.ts

    # ---------------- persistent SBUF ----------------
    _ctx = contextlib.ExitStack()
    persist = _ctx.enter_context(tc.tile_pool(name="persist", bufs=1))

    def ptile(shape, dt, tag):
        return persist.tile(shape, dt, tag=tag, name=tag)

    drampool = _ctx.enter_context(tc.tile_pool(name="drampool", bufs=1, space="DRAM"))

    weff_sb = ptile([P, 3, DK, P], bf16, "weff_sb")
    qt_sb = ptile([EPC, S], bf16, "qt_sb")
    kt_sb = ptile([EPC, S], bf16, "kt_sb")
    vaug_sb = ptile([P, HPC, TT, DQ + 1], bf16, "vaug_sb")
    ot_sb = ptile([EPC, S], bf16, "ot_sb")
    mask_sb = ptile([P, HPC, MW], bf16, "mask_sb")
    onesb_sb = ptile([P, 1], bf16, "onesb_sb")
    ident_sb = ptile([P, P], fp32, "ident_sb")
    identb_sb = ptile([P, P], bf16, "identb_sb")
    gb_sb = ptile([P, 6, DK], fp32, "gb_sb")     # g1,b1,g2,b2,l2b,(pad)
    l1b_sb = ptile([P, MK], fp32, "l1b_sb")
    x1t_sb = ptile([P, DK, SL], f32r, "x1t_sb")
    x1b_sb = ptile([P, DK, SL], bf16, "x1b_sb")   # bf16 twin of x1 for l1 matmul
    att_sb = ptile([P, DK, SL], bf16, "att_sb")
    mlp_sb = ptile([P, DK, SL], bf16, "mlp_sb")
    ots_sb = ptile([P, DK, SL], bf16, "ots_sb")
    wot_sb = ptile([P, DK, D], bf16, "wot_sb")
    l1t_sb = ptile([P, DK, DM], bf16, "l1t_sb")
    eres_sb = ptile([P, DK, SL], fp32, "eres_sb")

    # ---------------- constants (no DMA) ----------------
    nc.gpsimd.memset(onesb_sb[:], 1.0)
    make_identity(nc, ident_sb[:])
    nc.vector.tensor_copy(identb_sb[:], ident_sb[:])
    # causal diagonal mask, shared by both heads:
    #   mask[p, h, c] = 1 if c - p - (MW - QB) >= 0 else 0
    # For key tile t in query block qb (r = t - KPQ*qb), the [P, QB] mask is
    # the slice [:, :, (KPQ-1-r)*P : ...+QB]  <=>  keep iff qf >= kp + r*P.
    nc.gpsimd.memset(mask_sb[:], 1.0)
    nc.gpsimd.affine_select(
        out=mask_sb[:], in_=mask_sb[:], compare_op=mybir.AluOpType.is_ge,
        fill=0.0, base=-(MW - QB), channel_multiplier=-1,
        pattern=[[0, HPC], [1, MW]])
    # ones column of vaug (softmax denominator rows)
    nc.gpsimd.memset(vaug_sb[:, :, :, DQ:DQ + 1], 1.0)

    # ---------------- weight/activation DMAs, issued in need-order ----------
    etctx = contextlib.ExitStack()
    etpool = etctx.enter_context(tc.tile_pool(name="etpool", bufs=1))
    wctx = contextlib.ExitStack()
    htpool = wctx.enter_context(tc.tile_pool(name="htpool", bufs=1))
    wpool = wctx.enter_context(tc.tile_pool(name="wpool", bufs=3))

    ht_t = htpool.tile([P, DK, 3 * EPC], bf16, tag="ht_t")   # [p, kk, (j e)]
    nc.sync.dma_start(ht_t[:], io["hts"].rearrange("(kk p) e -> p kk e", p=P))

    def wload(wn):
        halves = []
        for half in range(2):
            wt = wpool.tile([P, DK // 2, D], bf16, tag="w_t")
            nc.sync.dma_start(
                wt[:], io[wn][half * 512:(half + 1) * 512, :]
                .rearrange("(kk p) d -> p kk d", p=P))
            halves.append(wt)
        return halves

    wq_h = wload("wq")
    eta_sb = etpool.tile([P, DK // 2, S], bf16, tag="eta_sb")
    nc.sync.dma_start(eta_sb[:], io["eT"][0:512, :].rearrange("(g p) s -> p g s", p=P))
    wk_h = wload("wk")
    etb_sb = etpool.tile([P, DK // 2, S], bf16, tag="etb_sb")
    nc.sync.dma_start(etb_sb[:], io["eT"][512:1024, :].rearrange("(g p) s -> p g s", p=P))
    wv_h = wload("wv")
    nc.sync.dma_start(eres_sb[:], io["eresT"].rearrange("(g p) s -> p g s", p=P))
    nc.sync.dma_start(wot_sb[:], io["wot"].rearrange("(g p) d -> p g d", p=P))
    for i, name in enumerate(("g1", "b1", "g2", "b2", "l2b")):
        nc.sync.dma_start(gb_sb[:, i, :], io[name].rearrange("(g p) -> p g", p=P))
    nc.sync.dma_start(l1b_sb[:], io["l1b"].rearrange("(g p) -> p g", p=P))
    for mg in range(4):
        nc.sync.dma_start(
            l1t_sb[:, :, ts(mg, 1024)],
            io["l1t"][:, ts(mg, 1024)].rearrange("(g p) m -> p g m", p=P))

    def et(kk):
        return (eta_sb if kk < 4 else etb_sb)[:, kk % 4, :]

    # ---------------- weff_j = (H_j @ W_j)^T, laid out [din, e] ----------------
    wedpool = wctx.enter_context(tc.tile_pool(name="wedpool", bufs=1))
    wpsum = wctx.enter_context(tc.tile_pool(name="wpsum", bufs=1, space="PSUM"))
    wtpsum = wctx.enter_context(tc.tile_pool(name="wtpsum", bufs=2, space="PSUM"))

    def weff_compute(j, w_halves, scale):
        psw = wpsum.tile([EPC, D], fp32, tag="psw")
        for half in range(2):
            for kk in range(4):
                k = half * 4 + kk
                for dh in range(2):
                    nc.tensor.matmul(psw[:, ts(dh, 512)], ht_t[:, k, ts(j, EPC)],
                                     w_halves[half][:, kk, ts(dh, 512)],
                                     start=(k == 0), stop=(k == 7))
        wed = wedpool.tile([EPC, D], bf16, tag="wed")
        nc.scalar.activation(wed[:], psw[:], Copy, scale=scale)
        for m in range(DK):
            pst = wtpsum.tile([P, P], bf16, tag="pst")
            nc.tensor.transpose(pst[:], wed[:, ts(m, P)], identb_sb[:])
            nc.scalar.activation(weff_sb[:, j, m, :], pst[:], Copy)

    # ---------------- qT, kT, v (interleaved with weff for DMA overlap) -------
    qkctx = contextlib.ExitStack()
    qkpsum = qkctx.enter_context(tc.tile_pool(name="qkpsum", bufs=1, space="PSUM"))

    weff_compute(0, wq_h, 1.0 / np.sqrt(DQ))
    psq = [qkpsum.tile([EPC, QB], fp32, tag=f"psq{sp}", name=f"psq{sp}")
           for sp in range(NQB)]
    for kk in range(4):                      # first half: needs only eta
        for sp in range(NQB):
            nc.tensor.matmul(psq[sp][:], weff_sb[:, 0, kk, :], et(kk)[:, ts(sp, QB)],
                             start=(kk == 0), stop=False)
    weff_compute(1, wk_h, 1.0)
    for kk in range(4, 8):
        for sp in range(NQB):
            nc.tensor.matmul(psq[sp][:], weff_sb[:, 0, kk, :], et(kk)[:, ts(sp, QB)],
                             start=False, stop=(kk == 7))
    for sp in range(NQB):
        nc.scalar.activation(qt_sb[:, ts(sp, QB)], psq[sp][:], Copy)
    qkctx.close()
    kpsum = wctx.enter_context(tc.tile_pool(name="kpsum", bufs=2, space="PSUM"))
    vtpool = wctx.enter_context(tc.tile_pool(name="vtpool", bufs=2))
    vtpsum = wctx.enter_context(tc.tile_pool(name="vtpsum", bufs=2, space="PSUM"))
    for sp in range(NQB):
        psk = kpsum.tile([EPC, QB], fp32, tag="psk")
        for kk in range(DK):
            nc.tensor.matmul(psk[:], weff_sb[:, 1, kk, :], et(kk)[:, ts(sp, QB)],
                             start=(kk == 0), stop=(kk == DK - 1))
        nc.scalar.activation(kt_sb[:, ts(sp, QB)], psk[:], Copy)
    weff_compute(2, wv_h, 1.0)
    for sp in range(NQB):
        psv = kpsum.tile([EPC, QB], fp32, tag="psk")
        for kk in range(DK):
            nc.tensor.matmul(psv[:], weff_sb[:, 2, kk, :], et(kk)[:, ts(sp, QB)],
                             start=(kk == 0), stop=(kk == DK - 1))
        vt_sp = vtpool.tile([EPC, QB], bf16, tag="vt_sp")
        nc.scalar.activation(vt_sp[:], psv[:], Copy)
        for tq in range(KPQ):
            t = sp * KPQ + tq
            pst = vtpsum.tile([P, P], bf16, tag="pstv")
            nc.tensor.transpose(pst[:], vt_sp[:, ts(tq, P)], identb_sb[:])
            # [128, (2h x 64e)] -> vaug[:, h, t, 0:64]
            nc.scalar.activation(
                vaug_sb[:, :, t, 0:DQ],
                pst[:].rearrange("s (h e) -> s h e", h=HPC), Copy)
    wctx.close()
    etctx.close()
    if stop_after == "qkv":
        _ctx.close(); return

    # ---------------- attention (causal skip, transposed scores) -------------
    with tc.tile_pool(name="scpsum", bufs=2, space="PSUM") as scpsum, \
         tc.tile_pool(name="opsum", bufs=2, space="PSUM") as opsum, \
         tc.tile_pool(name="expool", bufs=3) as expool, \
         tc.tile_pool(name="nrmpool", bufs=2) as nrmpool:
        for qb in range(NQB):
            TTq = KPQ * (qb + 1)             # key tiles at/below this q block
            qsl = slice(qb * QB, (qb + 1) * QB)
            pso = [opsum.tile([DQ + 1, QB], fp32, tag=f"pso{h}", name=f"pso{h}")
                   for h in range(HPC)]
            for t in range(TTq):
                ps2 = scpsum.tile([P, HPC, QB], fp32, tag="ps2")
                for h in range(HPC):
                    hs = slice(h * DQ, (h + 1) * DQ)
                    nc.tensor.matmul(ps2[:, h, :], kt_sb[hs, ts(t, P)],
                                     qt_sb[hs, qsl], start=True, stop=True)
                ex2 = expool.tile([P, HPC, QB], bf16, tag="ex2")
                nc.scalar.activation(ex2[:], ps2[:], Exp)
                r = t - KPQ * qb
                if r >= 0:                   # diagonal tile: mask (both heads)
                    off = (KPQ - 1 - r) * P
                    nc.vector.tensor_mul(ex2[:], ex2[:], mask_sb[:, :, off:off + QB])
                for h in range(HPC):
                    nc.tensor.matmul(pso[h][:], vaug_sb[:, h, t, :], ex2[:, h, :],
                                     start=(t == 0), stop=(t == TTq - 1))
            for h in range(HPC):
                hs = slice(h * DQ, (h + 1) * DQ)
                rc = nrmpool.tile([1, QB], bf16, tag="rc")
                with nc.allow_low_precision(reason="softmax denom; per-token scale cancels in LN1"):
                    nc.vector.reciprocal(rc[:], pso[h][DQ:DQ + 1, :])
                rcb = nrmpool.tile([DQ, QB], bf16, tag="rcb")
                nc.gpsimd.partition_broadcast(rcb[:], rc[:], channels=DQ)
                nc.vector.tensor_mul(ot_sb[hs, qsl], pso[h][:DQ, :], rcb[:])
    if stop_after == "attn":
        _ctx.close(); return

    # ---------------- AllToAll: heads-sharded -> token-sharded ----------------
    a2a_in = drampool.tile([NC * EPC, SL], bf16, tag="a2a_in", name="a2a_in")
    a2a_out = drampool.tile([NC * EPC, SL], bf16, tag="a2a_out", name="a2a_out")
    nc.sync.dma_start(a2a_in[:, :].rearrange("(g e) s -> e g s", e=EPC),
                      ot_sb[:].rearrange("e (g s) -> e g s", g=NC))
    if use_collective:
        import concourse.mybir as mybir2
        nc.gpsimd.collective_compute(
            "AllToAll", mybir2.AluOpType.bypass,
            replica_groups=[list(range(NC))],
            ins=[a2a_in.opt()], outs=[a2a_out.opt()])
    else:  # timing-only single-core variant (results wrong across cores)
        nc.sync.dma_start(a2a_out[:], a2a_in[:])
    nc.sync.dma_start(ots_sb[:], a2a_out[:, :].rearrange("(g e) s -> e g s", e=EPC))

    # ---------------- W_O projection ----------------
    with tc.tile_pool(name="wopsum", bufs=4, space="PSUM") as wopsum:
        for m in range(DK):
            psa = wopsum.tile([P, SL], fp32, tag="psa")
            for g in range(DK):
                nc.tensor.matmul(psa[:], wot_sb[:, g, ts(m, P)], ots_sb[:, g, :],
                                 start=(g == 0), stop=(g == DK - 1))
            nc.scalar.activation(att_sb[:, m, :], psa[:], Copy)
    if stop_after == "wo":
        _ctx.close(); return

    # ---------------- LayerNorm helper (stats over partition axis) -----------
    def layer_norm_T(src_sb, dst_sb, gi, bi, res_sb, pools, dst_bf=None):
        """dst = res + gamma*(src-mean)/sqrt(var+eps) + beta.
        src [P, DK, SL] bf16; res/dst fp32-class."""
        sqpool, stpsum, stpool = pools
        sq = sqpool.tile([P, DK, SL], bf16, tag="sq")
        nc.vector.tensor_mul(sq[:], src_sb[:], src_sb[:])
        ps1 = stpsum.tile([1, SL], fp32, tag="ps1")
        ps2 = stpsum.tile([1, SL], fp32, tag="ps2")
        for g in range(DK):
            nc.tensor.matmul(ps1[:], onesb_sb[:], src_sb[:, g, :],
                             start=(g == 0), stop=(g == DK - 1))
            nc.tensor.matmul(ps2[:], onesb_sb[:], sq[:, g, :],
                             start=(g == 0), stop=(g == DK - 1))
        mean = stpool.tile([1, SL], fp32, tag="mean")
        nc.vector.tensor_scalar_mul(mean[:], ps1[:], 1.0 / D)
        msq = stpool.tile([1, SL], fp32, tag="msq")
        nc.vector.tensor_mul(msq[:], mean[:], mean[:])
        var = stpool.tile([1, SL], fp32, tag="var")
        nc.vector.tensor_scalar_mul(var[:], ps2[:], 1.0 / D)
        nc.vector.tensor_tensor(var[:], var[:], msq[:], sub_op)
        nc.vector.tensor_scalar_add(var[:], var[:], EPS)
        std = stpool.tile([1, SL], fp32, tag="std")
        nc.scalar.activation(std[:], var[:], Sqrt)
        rstd = stpool.tile([1, SL], fp32, tag="rstd")
        nc.vector.reciprocal(rstd[:], std[:])
        mrs = stpool.tile([1, SL], bf16, tag="mrs")
        nc.vector.tensor_mul(mrs[:], mean[:], rstd[:])
        rstd_b = stpool.tile([1, SL], bf16, tag="rstd_b")
        nc.vector.tensor_copy(rstd_b[:], rstd[:])
        rstdb = stpool.tile([P, SL], bf16, tag="rstdb")
        nc.gpsimd.partition_broadcast(rstdb[:], rstd_b[:])
        mrsb = stpool.tile([P, SL], bf16, tag="mrsb")
        nc.gpsimd.partition_broadcast(mrsb[:], mrs[:])
        for g in range(DK):
            t1 = sqpool.tile([P, SL], bf16, tag="t1")
            nc.vector.tensor_mul(t1[:], src_sb[:, g, :], rstdb[:])
            nc.vector.tensor_tensor(t1[:], t1[:], mrsb[:], sub_op)
            t2 = sqpool.tile([P, SL], bf16, tag="t2")
            nc.scalar.activation(t2[:], t1[:], Ident,
                                 bias=gb_sb[:, bi, g:g + 1], scale=gb_sb[:, gi, g:g + 1])
            nc.gpsimd.tensor_tensor(dst_sb[:, g, :], t2[:], res_sb[:, g, :], add_op)
            if dst_bf is not None:
                nc.vector.tensor_tensor(dst_bf[:, g, :], t2[:], res_sb[:, g, :], add_op)

    with tc.tile_pool(name="sqpool", bufs=3) as sqpool, \
         tc.tile_pool(name="stpsum", bufs=2, space="PSUM") as stpsum, \
         tc.tile_pool(name="stpool", bufs=2) as stpool:
        layer_norm_T(att_sb, x1t_sb, 0, 1, eres_sb, (sqpool, stpsum, stpool),
                     dst_bf=x1b_sb)
    if stop_after == "ln1":
        _ctx.close(); return

    # ---------------- MLP ----------------
    with tc.tile_pool(name="hallpool", bufs=1) as hallpool, \
         tc.tile_pool(name="hpsum", bufs=4, space="PSUM") as hpsum:
        ht_all = hallpool.tile([P, MK, SL], bf16, tag="ht_all")
        for mc in range(MK):
            psh = hpsum.tile([P, SL], fp32, tag="psh")
            for g in range(DK):
                nc.tensor.matmul(psh[:], l1t_sb[:, g, ts(mc, P)], x1b_sb[:, g, :],
                                 start=(g == 0), stop=(g == DK - 1))
            nc.scalar.activation(ht_all[:, mc, :], psh[:], Relu,
                                 bias=l1b_sb[:, mc:mc + 1])
        with tc.tile_pool(name="l2pool", bufs=3) as l2pool, \
             tc.tile_pool(name="mlppsum", bufs=2, space="PSUM") as mlppsum:
            for dt_ in range(DK):
                l2td = l2pool.tile([P, MK, P], bf16, tag="l2td")
                nc.sync.dma_start(l2td[:], io["l2t"][dt_])
                psm2 = mlppsum.tile([P, SL], fp32, tag="psm2")
                for mc in range(MK):
                    nc.tensor.matmul(psm2[:], l2td[:, mc, :], ht_all[:, mc, :],
                                     start=(mc == 0), stop=(mc == MK - 1))
                nc.scalar.activation(mlp_sb[:, dt_, :], psm2[:], Ident,
                                     bias=gb_sb[:, 4, dt_:dt_ + 1])
    if stop_after == "mlp":
        _ctx.close(); return

    # ---------------- LN2 + residual; output stays [d, s] ----------------
    with tc.tile_pool(name="sqpool2", bufs=3) as sqpool2, \
         tc.tile_pool(name="stpsum2", bufs=2, space="PSUM") as stpsum2, \
         tc.tile_pool(name="stpool2", bufs=2) as stpool2:
        outT = sqpool2.tile([P, DK, SL], f32r, tag="outT")
        layer_norm_T(mlp_sb, outT, 2, 3, x1t_sb, (sqpool2, stpsum2, stpool2))
        nc.sync.dma_start(io["out"].rearrange("(g p) s -> p g s", p=P),
                          outT[:].bitcast(fp32))
    _ctx.close()


def build_program(S=S_FULL, use_collective=True, stop_after=None):
    import concourse.mybir as mybir
    import concourse.tile as tile
    from concourse import bacc

    nc = bacc.Bacc("TRN2", target_bir_lowering=False, debug=False,
                   enable_asserts=True, num_devices=NC if use_collective else 1)
    f32, bf16 = mybir.dt.float32, mybir.dt.bfloat16
    SL = S // NC

    def din(name, shape, dt=bf16):
        return nc.dram_tensor(name, shape, dt, kind="ExternalInput").ap()

    io = {
        "eT": din("eT", [D, S]),
        "eresT": din("eresT", [D, SL], f32),
        "wq": din("wq", [D, D]), "wk": din("wk", [D, D]),
        "wv": din("wv", [D, D]),
        "hts": din("hts", [D, 3 * EPC]),
        "wot": din("wot", [D, D]),
        "l1t": din("l1t", [D, DM]), "l2t": din("l2t", [DK, P, MK, P]),
        "l1b": din("l1b", [DM], f32), "l2b": din("l2b", [D], f32),
        "g1": din("g1", [D], f32), "b1": din("b1", [D], f32),
        "g2": din("g2", [D], f32), "b2": din("b2", [D], f32),
        "out": nc.dram_tensor("out", [D, SL], f32, kind="ExternalOutput").ap(),
    }
    with tile.TileContext(nc) as tc:
        _body(tc, io, S, use_collective, stop_after)
    nc.compile()
    return nc


def make_in_maps(E, mask, W_Q, W_K, W_V, W_O, H_Q, H_K, H_V,
                 L1_w, L1_b, L2_w, L2_b, gamma1, beta1, gamma2, beta2, S=S_FULL):
    E = np.asarray(E, np.float32).reshape(S, D)
    SL = S // NC
    eT = np.ascontiguousarray(E.T).astype(BF16)
    com = {
        "eT": eT,
        "wq": np.asarray(W_Q, np.float32).astype(BF16),
        "wk": np.asarray(W_K, np.float32).astype(BF16),
        "wv": np.asarray(W_V, np.float32).astype(BF16),
        "wot": np.ascontiguousarray(np.asarray(W_O, np.float32).T).astype(BF16),
        "l1t": np.ascontiguousarray(np.asarray(L1_w, np.float32).T).astype(BF16),
        "l2t": np.ascontiguousarray(
            np.asarray(L2_w, np.float32).T.reshape(MK, P, DK, P).transpose(2, 1, 0, 3)
        ).astype(BF16),
        "l1b": np.asarray(L1_b, np.float32), "l2b": np.asarray(L2_b, np.float32),
        "g1": np.asarray(gamma1, np.float32), "b1": np.asarray(beta1, np.float32),
        "g2": np.asarray(gamma2, np.float32), "b2": np.asarray(beta2, np.float32),
    }
    in_maps = []
    for c in range(NC):
        m = dict(com)
        m["eresT"] = np.ascontiguousarray(E[c * SL:(c + 1) * SL, :].T)
        hts = [np.asarray(Hw, np.float32)[c * HPC:(c + 1) * HPC].reshape(EPC, D).T
               for Hw in (H_Q, H_K, H_V)]
        m["hts"] = np.ascontiguousarray(np.concatenate(hts, axis=1)).astype(BF16)
        in_maps.append(m)
    return in_maps


_PROGRAM_CACHE = {}


def kernel(**inputs):
    from concourse import bass_utils
    S = inputs["E"].shape[1]
    if S not in _PROGRAM_CACHE:
        _PROGRAM_CACHE[S] = build_program(S)
    nc = _PROGRAM_CACHE[S]
    in_maps = make_in_maps(S=S, **inputs)
    res = bass_utils.run_bass_kernel_spmd(nc, in_maps, core_ids=list(range(NC)))
    SL = S // NC
    out = np.concatenate([np.ascontiguousarray(res.results[c]["out"].T)
                          for c in range(NC)], axis=0)
    return out.reshape(1, S, D).astype(np.float32)
